# revision 3
# baseline (speedup 1.0000x reference)
"""Trainium2 Bass kernel for nn_ClusterMemory (scatter_memory).

Reference computation (B=256, D=2048, S=65536, TEMP=0.05):
    x = inputs / ||inputs||_row            # [B, D]
    logits = (x @ features.T) / TEMP       # [B, S]
    loss = mean_i( logsumexp(logits[i,:]) - logits[i, targets[i]] )

Both x rows and features rows are L2-normalized, so every logit is a
cosine / TEMP, bounded to [-20, 20] -> exp() never overflows in f32 and no
max-subtraction pass is needed.  Each of the 8 cores returns the 16
per-j-chunk partial sums S_part[i, jc] = sum_j exp(logits[i, j]) over its
8192-row shard of the memory bank (features sharded row-wise).  The final
combine (sum the 128 partials per item, add the target-logit term) is done
on host in f64.

Inputs are scaled by 2^6 and cast to e4m3 on host; the PE runs fp8
DoubleRow (2 MACs/cell/cycle), which is the TRN2 matmul throughput ceiling
for this shape (N=512 moving, K=2048 contraction).  The kernel is
PE-bound: 256 DoubleRow matmuls x 512 columns = 131072 PE cycles ~ 55 us
at the warm 2.4 GHz clock, while the fp8 DMA stream (16.5 MB) sustains
>400 GB/s and finishes earlier.

v2 structure (from NTFF trace analysis of v1 @ 79.0 us):
  - The whole fp8 shard stays resident in SBUF ([128, 16, 16, 512], 128
    KB/partition) and is filled slice-wise by independent DMAs -> no tile
    pool recycling, fewer semaphores.
  - v1 lost 12 us before the first matmul: chunk0's sixteen sub-DMAs
    issued serially on the Sync queue (~610 ns each) and xT was one
    monolithic late transfer the first LDWEIGHTS had to wait out.  v2
    splits xT and chunk0 by k-pair groups and issues the pieces across
    BOTH HWDGE rings (sync + scalar) in consumption order.
  - Matmul order per chunk is k-pair-major with batch-half inner, so each
    newly arrived k-pair piece is consumed by two back-to-back matmuls.
  - 10 warm-up matmuls on scratch SBUF run during the DMA fill to lift
    the PE HAM clock gate (1.2 -> 2.4 GHz) before real work arrives.
  - No on-device reduction: the 16 partial sums per (item, half) DMA out
    as [128, 2, 16] f32 and the host adds them.
"""

import numpy as np

import concourse.bacc as bacc
import concourse.bass as bass
import concourse.mybir as mybir
import concourse.tile as tile

B = 256
D = 2048
S = 65536
TEMP = 0.05
N_CORES = 8
SHARD = S // N_CORES          # 8192 rows of the memory bank per core
JC = 512                      # j-chunk width (one PSUM bank of f32)
N_CHUNKS = SHARD // JC        # 16
KT = D // 128                 # 16 k-tiles of 128
KP = KT // 2                  # 8 DoubleRow k-pairs

MODE = "fp8"                  # v2 is fp8-only (PE + DMA optimal)

# e4m3 normal range starts at 2^-6; x/feats components are ~N(0, 1/2048)
# (sigma 0.022), so scale by 2^6 to keep ~99% of them normal.  The matmul
# then computes (64x)·(64f); the 1/4096 is folded into the ACT exp scale.
FP8_SCALE = 64.0


def build_nc(mode=MODE):
    assert mode == "fp8", "v2 kernel only supports fp8 mode"
    f32 = mybir.dt.float32
    in_dt = mybir.dt.float8e4
    act_scale = (1.0 / TEMP) / (FP8_SCALE * FP8_SCALE)
    DR = mybir.MatmulPerfMode.DoubleRow

    nc = bacc.Bacc("TRN2", target_bir_lowering=False, debug=False,
                   num_devices=N_CORES)
    xT_d = nc.dram_tensor("xT", [128, KT, B], in_dt, kind="ExternalInput")
    featsT_d = nc.dram_tensor("featsT", [N_CHUNKS, 128, KT, JC], in_dt,
                              kind="ExternalInput")
    s_d = nc.dram_tensor("s_out", [128, 2, N_CHUNKS], f32,
                         kind="ExternalOutput")

    with tile.TileContext(nc) as tc:
        with (
            tc.tile_pool(name="data", bufs=1) as dpool,
            tc.tile_pool(name="psum", bufs=7, space="PSUM") as ppool,
        ):
            # Resident tiles (bufs=1 -> no recycling semaphores).
            feats = dpool.tile([128, N_CHUNKS, KT, JC], in_dt)
            xT = dpool.tile([128, KT, B], in_dt)
            sums = dpool.tile([128, 2, N_CHUNKS], f32)
            junk = dpool.tile([128, JC], f32)
            warm_x = dpool.tile([128, 2, 128], in_dt)
            warm_f = dpool.tile([128, 2, 128], in_dt)

            # PE warm-up: scratch matmuls with no DMA dependencies keep the
            # PE busy from the end of the framework prologue so the HAM
            # clock gate reaches 8/8 before the real stream begins.
            nc.gpsimd.memset(warm_x[:], 0.0)
            nc.gpsimd.memset(warm_f[:], 0.0)
            warm_ps = ppool.tile([128, 128], f32, tag="ps", name="warm_ps")
            for _ in range(10):
                nc.tensor.matmul(warm_ps[:], warm_x[:], warm_f[:],
                                 start=True, stop=True, perf_mode=DR,
                                 skip_group_check=True)

            # Startup DMAs, split by k-pair group and spread across both
            # HWDGE rings (sync = qSPDynamicHW, scalar = qActDynamicHW) in
            # consumption order.  The scalar ring starts ~1.3 us late (the
            # framework's ACT table load precedes ours there), so the
            # first-needed pieces ride sync.
            nc.sync.dma_start(out=xT[:, 0:4, :], in_=xT_d[:, 0:4, :])
            nc.sync.dma_start(out=feats[:, 0, 0:4, :],
                              in_=featsT_d[0, :, 0:4, :])
            nc.scalar.dma_start(out=feats[:, 0, 4:8, :],
                                in_=featsT_d[0, :, 4:8, :])
            nc.sync.dma_start(out=xT[:, 4:16, :], in_=xT_d[:, 4:16, :])
            nc.scalar.dma_start(out=feats[:, 0, 8:16, :],
                                in_=featsT_d[0, :, 8:16, :])
            # Remaining chunks alternate rings.
            for c in range(1, N_CHUNKS):
                eng = nc.sync if c % 2 == 1 else nc.scalar
                eng.dma_start(out=feats[:, c], in_=featsT_d[c])

            # Main PE stream: per chunk, k-pair-major with batch-half
            # inner; the two interleaved PSUM accumulation groups (one per
            # batch half) each see start on kp 0 and stop on kp 7.
            for c in range(N_CHUNKS):
                ps = [ppool.tile([128, JC], f32, tag="ps", name="ps")
                      for _ in range(2)]
                for t in range(KP):
                    for bh in range(2):
                        bsl = slice(bh * 128, (bh + 1) * 128)
                        nc.tensor.matmul(
                            ps[bh][:],
                            xT[:, 2 * t:2 * t + 2, bsl],
                            feats[:, c, 2 * t:2 * t + 2, :],
                            start=(t == 0), stop=(t == KP - 1),
                            perf_mode=DR, skip_group_check=True)
                for bh in range(2):
                    nc.scalar.activation(
                        junk[:], ps[bh][:], mybir.ActivationFunctionType.Exp,
                        scale=act_scale,
                        accum_out=sums[:, bh, c:c + 1])

            nc.sync.dma_start(out=s_d[:], in_=sums[:])

    nc.compile()
    return nc


_NC_CACHE = {}


def _get_nc(mode=MODE):
    if mode not in _NC_CACHE:
        _NC_CACHE[mode] = build_nc(mode)
    return _NC_CACHE[mode]


def host_prep(inputs, features, mode=MODE):
    """Normalize/transpose/pack on host; returns (x_norm_f32, in_maps)."""
    import ml_dtypes
    x = np.asarray(inputs, dtype=np.float32)
    x = x / np.linalg.norm(x, axis=1, keepdims=True)
    np_dt = ml_dtypes.float8_e4m3
    scale = np.float32(FP8_SCALE)

    # xT[p, kk, b] = x[b, kk*128 + p]
    xT = np.ascontiguousarray(
        (x.T * scale).reshape(KT, 128, B).transpose(1, 0, 2).astype(np_dt))

    feats = np.asarray(features, dtype=np.float32) * scale
    in_maps = []
    for c in range(N_CORES):
        # shardT[k, j] = feats[c*SHARD + j, k]; packed[jc, p, kk, j] =
        # shardT[kk*128 + p, jc*JC + j]
        shardT = feats[c * SHARD:(c + 1) * SHARD].T       # [D, SHARD] view
        packed = np.ascontiguousarray(
            shardT.reshape(KT, 128, N_CHUNKS, JC).transpose(2, 1, 0, 3)
            .astype(np_dt))
        in_maps.append({"xT": xT, "featsT": packed})
    return x, in_maps


def combine(x, features, targets, core_outs):
    """Host combine: sum shard partials, add the target-logit term."""
    S_total = np.zeros(B, dtype=np.float64)
    for out in core_outs:
        s = out["s_out"].astype(np.float64)       # [128, 2, N_CHUNKS]
        S_total += s.sum(axis=2).T.reshape(-1)    # item i = h*128 + p
    t = np.asarray(targets).astype(np.int64)
    f_t = np.asarray(features, dtype=np.float32)[t]          # [B, D]
    l_tgt = np.einsum("ij,ij->i", x.astype(np.float64),
                      f_t.astype(np.float64)) / TEMP
    loss = np.mean(np.log(S_total) - l_tgt)
    return np.array(loss, dtype=np.float32)


def kernel(**inputs):
    from concourse.bass_utils import run_bass_kernel_spmd

    x, in_maps = host_prep(inputs["inputs"], inputs["features"])
    nc = _get_nc()
    res = run_bass_kernel_spmd(nc, in_maps, list(range(N_CORES)))
    return combine(x, inputs["features"], inputs["targets"], res.results)
